# revision 11
# baseline (speedup 1.0000x reference)
"""Trainium2 Bass kernel for quantized Conv1dSubsampling (nn_Conv1dSubsampling).

Reference computation (per batch):
  xq  = fake_quant_act(x, s1, 8)                    # clamp +/- s1, round to 255-level grid
  y   = w1q @ xq                                    # 1x1 conv: [512,80] @ [80,T]
  yq  = fake_quant_act(y, s2, 8)
  out = depthwise_conv(yq, w2q, stride=4, k=8)      # [512, 2047]

Kernel strategy (8 cores, data-parallel over batch: 2 batches/core):
  - Exact integer formulation: quantized activations/weights are integer
    levels in [-127,127], exact in bf16.  Both matmuls run on the PE in bf16
    with exact fp32 PSUM accumulation (|sums| < 2^24).  Per-channel scales
    are applied in the PSUM-evacuation ops (per-partition scalar operands).
  - Rounding uses the fp32 magic constant C=1.5*2^23: fl(fl(v+C)-C) = RNE(v).
  - Depthwise conv = 8 PSUM-accumulating matmuls with diagonal weight
    matrices (tap k = r + 4j: out[:, t] += diag(W2i[:, k]) @ Z_r[:, t+j]) on
    time-deinterleaved data Z_r[c, u] = yq[c, 4u + r].  x arrives from the
    host pre-deinterleaved and pre-scaled, so every device op is contiguous.
"""

import ml_dtypes
import numpy as np

import concourse.bass as bass  # noqa: F401  (env import check)
import concourse.mybir as mybir
from concourse import bacc
from concourse import bass_utils
from concourse.tile import TileContext
from concourse.tile import add_dep_helper

# Problem shapes (hardcoded per contest contract).
B, CIN, T, COUT = 16, 80, 8192, 512
SF = 4
KW = 2 * SF                      # depthwise kernel width = 8
TOUT = (T - KW) // SF + 1        # 2047
N_CORES = 8
BPC = B // N_CORES               # batches per core = 2
EPS = 1e-5
GAMMA = 0.9
QMAX = 127.0
MAGIC = float(np.float32(1.5 * 2.0**23))  # 12582912.0
TD = T // SF                     # 2048 (deinterleaved length per residue)
NW = TD // 512                   # 4 windows of 512 per residue
F32 = mybir.dt.float32
BF16 = mybir.dt.bfloat16
ADD = mybir.AluOpType.add
MAX = mybir.AluOpType.max
MIN = mybir.AluOpType.min
MULT = mybir.AluOpType.mult

_COMPILED = None


def _quant_weight_int(w):
    """Integer levels + per-channel step of reference fake_quant_weight."""
    red = tuple(range(1, w.ndim))
    s = np.maximum(
        np.float32(GAMMA) * np.max(np.abs(w), axis=red, keepdims=True),
        np.float32(EPS),
    ).astype(np.float32)
    step = (s / np.float32(QMAX)).astype(np.float32)
    wc = np.clip(w, -s, s).astype(np.float32)
    wi = np.round((wc / step).astype(np.float32)).astype(np.float32)
    return wi, step.reshape(w.shape[0])


def _build_program():
    nc = bacc.Bacc(
        "TRN2",
        target_bir_lowering=False,
        debug=False,
        enable_asserts=False,
        num_devices=N_CORES,
    )

    xd = nc.dram_tensor("xd", [BPC, CIN, SF, TD], F32, kind="ExternalInput")
    w1t = nc.dram_tensor("w1t", [CIN, COUT], BF16, kind="ExternalInput")
    w2d = nc.dram_tensor("w2d", [128, 4 * KW * 128], BF16, kind="ExternalInput")
    beta = nc.dram_tensor("beta", [COUT], F32, kind="ExternalInput")
    gout = nc.dram_tensor("gout", [COUT], F32, kind="ExternalInput")
    outd = nc.dram_tensor("out", [BPC, COUT, TOUT], F32, kind="ExternalOutput")

    C = MAGIC
    pe_chain = []

    with TileContext(nc) as tc:
        with (
            tc.tile_pool(name="wpool", bufs=1) as wpool,
            tc.tile_pool(name="xraw", bufs=1) as xraw_pool,
            tc.tile_pool(name="xtmp", bufs=1) as xtmp_pool,
            tc.tile_pool(name="xip", bufs=2) as xi_pool,
            tc.tile_pool(name="yqp", bufs=2) as yq_pool,
            tc.tile_pool(name="qch", bufs=4) as qch_pool,
            tc.tile_pool(name="outp", bufs=3) as out_pool,
            tc.tile_pool(name="pmm", bufs=2, space="PSUM") as pmm_pool,
            tc.tile_pool(name="pdw", bufs=1, space="PSUM") as pdw_pool,
        ):
            w1sb = wpool.tile([CIN, COUT], BF16)          # lhsT for main mm
            nc.sync.dma_start(w1sb, w1t.ap())
            w2sb = wpool.tile([128, 4 * KW * 128], BF16)  # 32 diag blocks
            nc.sync.dma_start(w2sb, w2d.ap())
            beta_sb = wpool.tile([128, 4], F32)
            nc.sync.dma_start(beta_sb, beta.ap().rearrange("(q p) -> p q", p=128))
            gout_sb = wpool.tile([128, 4], F32)
            nc.sync.dma_start(gout_sb, gout.ap().rearrange("(q p) -> p q", p=128))

            for b in range(BPC):
                # x already scaled to u-units (x/step1) and deinterleaved on host
                xr = xraw_pool.tile([CIN, T], F32, tag="xr")
                nc.sync.dma_start(xr, xd.ap()[b].rearrange("c r u -> c (r u)"))
                xrnd = xtmp_pool.tile([CIN, T], F32, tag="xrnd")
                nc.vector.tensor_scalar(xrnd, xr, C, -C, ADD, ADD)  # round to int
                xi = xi_pool.tile([CIN, T], BF16, tag="xi")
                nc.vector.tensor_scalar(xi, xrnd, -QMAX, QMAX, MAX, MIN)

                for cb in range(4):
                    yq = yq_pool.tile([128, T], BF16, tag="yq")
                    for g in range(8):  # groups of 1024 (r = g//2, half = g%2)
                        r, h = g // 2, g % 2
                        base = r * TD + h * 1024
                        pu = pmm_pool.tile([128, 1024], F32, tag="pu")
                        for w in range(2):
                            mm = nc.tensor.matmul(
                                pu[:, w * 512 : (w + 1) * 512],
                                w1sb[:, cb * 128 : (cb + 1) * 128],
                                xi[:, base + w * 512 : base + (w + 1) * 512],
                                start=True,
                                stop=True,
                            )
                            pe_chain.append(mm.ins)
                        # u + C = S*beta + C  (single fused affine on ACT)
                        tq = qch_pool.tile([128, 1024], F32, tag="tq")
                        nc.scalar.activation(
                            tq,
                            pu,
                            mybir.ActivationFunctionType.Copy,
                            bias=C,
                            scale=beta_sb[:, cb : cb + 1],
                        )
                        # subtract C (round done) and clamp low
                        t2 = qch_pool.tile([128, 1024], BF16, tag="t2")
                        nc.vector.tensor_scalar(t2, tq, -C, -QMAX, ADD, MAX)
                        # clamp high -> integer levels of yq, in bf16
                        nc.vector.tensor_scalar(
                            yq[:, base : base + 1024], t2, QMAX, None, MIN
                        )

                    # depthwise conv: k-outer over 4 window banks so each diag
                    # weight load serves 4 consecutive matmuls (streams ~216ns)
                    pd = pdw_pool.tile([128, 2048], F32, tag="pd")
                    for k in range(KW):
                        r, j = k % SF, k // SF
                        for w4 in range(4):
                            n = 512 if w4 < 3 else TOUT - 3 * 512
                            base = r * TD + w4 * 512 + j
                            mm = nc.tensor.matmul(
                                pd[:, w4 * 512 : w4 * 512 + n],
                                w2sb[:, (cb * KW + k) * 128 : (cb * KW + k + 1) * 128],
                                yq[:, base : base + n],
                                start=(k == 0),
                                stop=(k == KW - 1),
                            )
                            pe_chain.append(mm.ins)
                    for ho in range(2):
                        n = 1024 if ho == 0 else TOUT - 1024
                        osb = out_pool.tile([128, 1024], F32, tag="osb")
                        if ho % 2 == 0:
                            nc.vector.tensor_scalar(
                                osb[:, :n],
                                pd[:, ho * 1024 : ho * 1024 + n],
                                gout_sb[:, cb : cb + 1],
                                None,
                                MULT,
                            )
                        else:
                            nc.scalar.activation(
                                osb[:, :n],
                                pd[:, ho * 1024 : ho * 1024 + n],
                                mybir.ActivationFunctionType.Copy,
                                bias=0.0,
                                scale=gout_sb[:, cb : cb + 1],
                            )
                        nc.sync.dma_start(
                            outd.ap()[
                                b,
                                cb * 128 : (cb + 1) * 128,
                                ho * 1024 : ho * 1024 + n,
                            ],
                            osb[:, :n],
                        )

        # force PE program order = emission order so same-weight matmul runs
        # stay consecutive (weight reloads pipeline; avoids isolated-MM cost)
        for i in range(1, len(pe_chain)):
            add_dep_helper(pe_chain[i], pe_chain[i - 1], sync=False,
                           reason="pe-order")

    nc.compile()
    return nc


def _get_program():
    global _COMPILED
    if _COMPILED is None:
        _COMPILED = _build_program()
    return _COMPILED


def kernel(x, x_len, w1, w2, s1, s2):
    out, _res = run_with_results(x, x_len, w1, w2, s1, s2)
    return out


def run_with_results(x, x_len, w1, w2, s1, s2, trace=False, **trace_kwargs):
    x = np.asarray(x, dtype=np.float32)
    x_len = np.asarray(x_len, dtype=np.float32)
    w1 = np.asarray(w1, dtype=np.float32)
    w2 = np.asarray(w2, dtype=np.float32)
    s1f = np.maximum(np.float32(np.asarray(s1).reshape(())), np.float32(EPS))
    s2f = np.maximum(np.float32(np.asarray(s2).reshape(())), np.float32(EPS))

    # host-side weight quantization (integer levels + scales)
    w1i, step_w1 = _quant_weight_int(w1)   # [512, 80, 1] ints, [512]
    w1i = w1i[:, :, 0]
    w2i, step_w2 = _quant_weight_int(w2)   # [512, 1, 8] ints, [512]
    w2i = w2i[:, 0, :]

    step1 = np.float32(s1f / np.float32(QMAX))
    step2 = np.float32(s2f / np.float32(QMAX))

    # beta: u = S * beta + rounding;  y = step_w1*step1*S ; u = y*127/s2
    beta = (step_w1 * step1 * (np.float32(QMAX) / s2f)).astype(np.float32)
    # gout: out = gout * sum_k W2i*yq_int
    gout = (step_w2 * step2).astype(np.float32)

    w1t = np.ascontiguousarray(w1i.T).astype(ml_dtypes.bfloat16)  # [80, 512]
    w2dm = np.zeros((128, 4 * KW * 128), dtype=np.float32)
    p = np.arange(128)
    for cb in range(4):
        for k in range(KW):
            w2dm[p, (cb * KW + k) * 128 + p] = w2i[cb * 128 + p, k]
    w2dm = w2dm.astype(ml_dtypes.bfloat16)

    # x -> u-units (divide, matching reference xc/step), deinterleave by residue
    xs = (x / step1).astype(np.float32)
    xd = np.ascontiguousarray(xs.reshape(B, CIN, TD, SF).transpose(0, 1, 3, 2))

    nc = _get_program()
    in_maps = []
    for core in range(N_CORES):
        in_maps.append(
            {
                "xd": xd[core * BPC : (core + 1) * BPC],
                "w1t": w1t,
                "w2d": w2dm,
                "beta": beta,
                "gout": gout,
            }
        )

    res = bass_utils.run_bass_kernel_spmd(
        nc, in_maps, core_ids=list(range(N_CORES)), trace=trace, **trace_kwargs
    )
    out = np.concatenate([r["out"] for r in res.results], axis=0)
    return (out, np.floor(x_len / SF)), res


# revision 12
# speedup vs baseline: 1.1971x; 1.1971x over previous
"""Trainium2 Bass kernel for quantized Conv1dSubsampling (nn_Conv1dSubsampling).

Reference computation (per batch):
  xq  = fake_quant_act(x, s1, 8)                    # clamp +/- s1, round to 255-level grid
  y   = w1q @ xq                                    # 1x1 conv: [512,80] @ [80,T]
  yq  = fake_quant_act(y, s2, 8)
  out = depthwise_conv(yq, w2q, stride=4, k=8)      # [512, 2047]

Kernel strategy (8 cores, data-parallel over batch: 2 batches/core):
  - Exact integer formulation: quantized activations/weights are integer
    levels in [-127,127], exact in bf16.  Both matmuls run on the PE in bf16
    with exact fp32 PSUM accumulation (|sums| < 2^24).  Per-channel scales
    are applied in the PSUM-evacuation ops (per-partition scalar operands).
  - Rounding uses the fp32 magic constant C=1.5*2^23: fl(fl(v+C)-C) = RNE(v).
  - Depthwise conv = 8 PSUM-accumulating matmuls with diagonal weight
    matrices (tap k = r + 4j: out[:, t] += diag(W2i[:, k]) @ Z_r[:, t+j]) on
    time-deinterleaved data Z_r[c, u] = yq[c, 4u + r].  x arrives from the
    host pre-deinterleaved and pre-scaled, so every device op is contiguous.
"""

import ml_dtypes
import numpy as np

import concourse.bass as bass  # noqa: F401  (env import check)
import concourse.mybir as mybir
from concourse import bacc
from concourse import bass_utils
from concourse.tile import TileContext
from concourse.tile import add_dep_helper

# Problem shapes (hardcoded per contest contract).
B, CIN, T, COUT = 16, 80, 8192, 512
SF = 4
KW = 2 * SF                      # depthwise kernel width = 8
TOUT = (T - KW) // SF + 1        # 2047
N_CORES = 8
BPC = B // N_CORES               # batches per core = 2
EPS = 1e-5
GAMMA = 0.9
QMAX = 127.0
MAGIC = float(np.float32(1.5 * 2.0**23))  # 12582912.0
TD = T // SF                     # 2048 (deinterleaved length per residue)
NW = TD // 512                   # 4 windows of 512 per residue
F32 = mybir.dt.float32
BF16 = mybir.dt.bfloat16
ADD = mybir.AluOpType.add
MAX = mybir.AluOpType.max
MIN = mybir.AluOpType.min
MULT = mybir.AluOpType.mult

_COMPILED = None


def _quant_weight_int(w):
    """Integer levels + per-channel step of reference fake_quant_weight."""
    red = tuple(range(1, w.ndim))
    s = np.maximum(
        np.float32(GAMMA) * np.max(np.abs(w), axis=red, keepdims=True),
        np.float32(EPS),
    ).astype(np.float32)
    step = (s / np.float32(QMAX)).astype(np.float32)
    wc = np.clip(w, -s, s).astype(np.float32)
    wi = np.round((wc / step).astype(np.float32)).astype(np.float32)
    return wi, step.reshape(w.shape[0])


def _build_program():
    nc = bacc.Bacc(
        "TRN2",
        target_bir_lowering=False,
        debug=False,
        enable_asserts=False,
        num_devices=N_CORES,
    )

    xd = nc.dram_tensor("xd", [BPC, CIN, SF, TD], F32, kind="ExternalInput")
    w1t = nc.dram_tensor("w1t", [CIN, COUT], BF16, kind="ExternalInput")
    w2d = nc.dram_tensor("w2d", [128, 4 * KW * 128], BF16, kind="ExternalInput")
    beta = nc.dram_tensor("beta", [COUT], F32, kind="ExternalInput")
    gout = nc.dram_tensor("gout", [COUT], F32, kind="ExternalInput")
    outd = nc.dram_tensor("out", [BPC, COUT, TOUT], F32, kind="ExternalOutput")

    C = MAGIC
    pe_chain = []

    with TileContext(nc) as tc:
        with (
            tc.tile_pool(name="wpool", bufs=1) as wpool,
            tc.tile_pool(name="xraw", bufs=1) as xraw_pool,
            tc.tile_pool(name="xtmp", bufs=1) as xtmp_pool,
            tc.tile_pool(name="xip", bufs=2) as xi_pool,
            tc.tile_pool(name="yqp", bufs=2) as yq_pool,
            tc.tile_pool(name="qch", bufs=4) as qch_pool,
            tc.tile_pool(name="outp", bufs=3) as out_pool,
            tc.tile_pool(name="pmm", bufs=2, space="PSUM") as pmm_pool,
            tc.tile_pool(name="pdw", bufs=2, space="PSUM") as pdw_pool,
        ):
            w1sb = wpool.tile([CIN, COUT], BF16)          # lhsT for main mm
            nc.sync.dma_start(w1sb, w1t.ap())
            w2sb = wpool.tile([128, 4 * KW * 128], BF16)  # 32 diag blocks
            nc.sync.dma_start(w2sb, w2d.ap())
            beta_sb = wpool.tile([128, 4], F32)
            nc.sync.dma_start(beta_sb, beta.ap().rearrange("(q p) -> p q", p=128))
            gout_sb = wpool.tile([128, 4], F32)
            nc.sync.dma_start(gout_sb, gout.ap().rearrange("(q p) -> p q", p=128))

            for b in range(BPC):
                # x already scaled to u-units (x/step1) and deinterleaved on host
                xr = xraw_pool.tile([CIN, T], F32, tag="xr")
                nc.sync.dma_start(xr, xd.ap()[b].rearrange("c r u -> c (r u)"))
                xrnd = xtmp_pool.tile([CIN, T], F32, tag="xrnd")
                nc.vector.tensor_scalar(xrnd, xr, C, -C, ADD, ADD)  # round to int
                xi = xi_pool.tile([CIN, T], BF16, tag="xi")
                nc.vector.tensor_scalar(xi, xrnd, -QMAX, QMAX, MAX, MIN)

                for cb in range(4):
                    yq = yq_pool.tile([128, T], BF16, tag="yq")
                    for g in range(8):  # groups of 1024 (r = g//2, half = g%2)
                        r, h = g // 2, g % 2
                        base = r * TD + h * 1024
                        pu = pmm_pool.tile([128, 1024], F32, tag="pu")
                        for w in range(2):
                            mm = nc.tensor.matmul(
                                pu[:, w * 512 : (w + 1) * 512],
                                w1sb[:, cb * 128 : (cb + 1) * 128],
                                xi[:, base + w * 512 : base + (w + 1) * 512],
                                start=True,
                                stop=True,
                            )
                            pe_chain.append(mm.ins)
                        # u + C = S*beta + C  (single fused affine on ACT)
                        tq = qch_pool.tile([128, 1024], F32, tag="tq")
                        nc.scalar.activation(
                            tq,
                            pu,
                            mybir.ActivationFunctionType.Copy,
                            bias=C,
                            scale=beta_sb[:, cb : cb + 1],
                        )
                        # subtract C (round done) and clamp low
                        t2 = qch_pool.tile([128, 1024], BF16, tag="t2")
                        nc.vector.tensor_scalar(t2, tq, -C, -QMAX, ADD, MAX)
                        # clamp high -> integer levels of yq, in bf16
                        nc.vector.tensor_scalar(
                            yq[:, base : base + 1024], t2, QMAX, None, MIN
                        )

                    # depthwise conv: window-pairs in double-buffered PSUM,
                    # k-outer within a pair (2 same-weight matmuls per load)
                    for ho in range(2):
                        n = 1024 if ho == 0 else TOUT - 1024
                        pd = pdw_pool.tile([128, 1024], F32, tag="pd")
                        for k in range(KW):
                            r, j = k % SF, k // SF
                            for w4 in range(2):
                                m = min(512, n - w4 * 512)
                                base = r * TD + (2 * ho + w4) * 512 + j
                                mm = nc.tensor.matmul(
                                    pd[:, w4 * 512 : w4 * 512 + m],
                                    w2sb[
                                        :,
                                        (cb * KW + k) * 128 : (cb * KW + k + 1) * 128,
                                    ],
                                    yq[:, base : base + m],
                                    start=(k == 0),
                                    stop=(k == KW - 1),
                                )
                                pe_chain.append(mm.ins)
                        osb = out_pool.tile([128, 1024], F32, tag="osb")
                        if ho % 2 == 0:
                            nc.vector.tensor_scalar(
                                osb[:, :n], pd[:, :n],
                                gout_sb[:, cb : cb + 1], None, MULT,
                            )
                        else:
                            nc.scalar.activation(
                                osb[:, :n], pd[:, :n],
                                mybir.ActivationFunctionType.Copy,
                                bias=0.0, scale=gout_sb[:, cb : cb + 1],
                            )
                        nc.sync.dma_start(
                            outd.ap()[
                                b,
                                cb * 128 : (cb + 1) * 128,
                                ho * 1024 : ho * 1024 + n,
                            ],
                            osb[:, :n],
                        )

        pass

    nc.compile()
    return nc


def _get_program():
    global _COMPILED
    if _COMPILED is None:
        _COMPILED = _build_program()
    return _COMPILED


def kernel(x, x_len, w1, w2, s1, s2):
    out, _res = run_with_results(x, x_len, w1, w2, s1, s2)
    return out


def run_with_results(x, x_len, w1, w2, s1, s2, trace=False, **trace_kwargs):
    x = np.asarray(x, dtype=np.float32)
    x_len = np.asarray(x_len, dtype=np.float32)
    w1 = np.asarray(w1, dtype=np.float32)
    w2 = np.asarray(w2, dtype=np.float32)
    s1f = np.maximum(np.float32(np.asarray(s1).reshape(())), np.float32(EPS))
    s2f = np.maximum(np.float32(np.asarray(s2).reshape(())), np.float32(EPS))

    # host-side weight quantization (integer levels + scales)
    w1i, step_w1 = _quant_weight_int(w1)   # [512, 80, 1] ints, [512]
    w1i = w1i[:, :, 0]
    w2i, step_w2 = _quant_weight_int(w2)   # [512, 1, 8] ints, [512]
    w2i = w2i[:, 0, :]

    step1 = np.float32(s1f / np.float32(QMAX))
    step2 = np.float32(s2f / np.float32(QMAX))

    # beta: u = S * beta + rounding;  y = step_w1*step1*S ; u = y*127/s2
    beta = (step_w1 * step1 * (np.float32(QMAX) / s2f)).astype(np.float32)
    # gout: out = gout * sum_k W2i*yq_int
    gout = (step_w2 * step2).astype(np.float32)

    w1t = np.ascontiguousarray(w1i.T).astype(ml_dtypes.bfloat16)  # [80, 512]
    w2dm = np.zeros((128, 4 * KW * 128), dtype=np.float32)
    p = np.arange(128)
    for cb in range(4):
        for k in range(KW):
            w2dm[p, (cb * KW + k) * 128 + p] = w2i[cb * 128 + p, k]
    w2dm = w2dm.astype(ml_dtypes.bfloat16)

    # x -> u-units (divide, matching reference xc/step), deinterleave by residue
    xs = (x / step1).astype(np.float32)
    xd = np.ascontiguousarray(xs.reshape(B, CIN, TD, SF).transpose(0, 1, 3, 2))

    nc = _get_program()
    in_maps = []
    for core in range(N_CORES):
        in_maps.append(
            {
                "xd": xd[core * BPC : (core + 1) * BPC],
                "w1t": w1t,
                "w2d": w2dm,
                "beta": beta,
                "gout": gout,
            }
        )

    res = bass_utils.run_bass_kernel_spmd(
        nc, in_maps, core_ids=list(range(N_CORES)), trace=trace, **trace_kwargs
    )
    out = np.concatenate([r["out"] for r in res.results], axis=0)
    return (out, np.floor(x_len / SF)), res


# revision 13
# speedup vs baseline: 1.2182x; 1.0176x over previous
"""Trainium2 Bass kernel for quantized Conv1dSubsampling (nn_Conv1dSubsampling).

Reference computation (per batch):
  xq  = fake_quant_act(x, s1, 8)                    # clamp +/- s1, round to 255-level grid
  y   = w1q @ xq                                    # 1x1 conv: [512,80] @ [80,T]
  yq  = fake_quant_act(y, s2, 8)
  out = depthwise_conv(yq, w2q, stride=4, k=8)      # [512, 2047]

Kernel strategy (8 cores, data-parallel over batch: 2 batches/core):
  - Exact integer formulation: quantized activations/weights are integer
    levels in [-127,127], exact in bf16.  Both matmuls run on the PE in bf16
    with exact fp32 PSUM accumulation (|sums| < 2^24).  Per-channel scales
    are applied in the PSUM-evacuation ops (per-partition scalar operands).
  - Rounding uses the fp32 magic constant C=1.5*2^23: fl(fl(v+C)-C) = RNE(v).
  - Depthwise conv = 8 PSUM-accumulating matmuls with diagonal weight
    matrices (tap k = r + 4j: out[:, t] += diag(W2i[:, k]) @ Z_r[:, t+j]) on
    time-deinterleaved data Z_r[c, u] = yq[c, 4u + r].  x arrives from the
    host pre-deinterleaved and pre-scaled, so every device op is contiguous.
"""

import ml_dtypes
import numpy as np

import concourse.bass as bass  # noqa: F401  (env import check)
import concourse.mybir as mybir
from concourse import bacc
from concourse import bass_utils
from concourse.tile import TileContext
from concourse.tile import add_dep_helper

# Problem shapes (hardcoded per contest contract).
B, CIN, T, COUT = 16, 80, 8192, 512
SF = 4
KW = 2 * SF                      # depthwise kernel width = 8
TOUT = (T - KW) // SF + 1        # 2047
N_CORES = 8
BPC = B // N_CORES               # batches per core = 2
EPS = 1e-5
GAMMA = 0.9
QMAX = 127.0
MAGIC = float(np.float32(1.5 * 2.0**23))  # 12582912.0
TD = T // SF                     # 2048 (deinterleaved length per residue)
NW = TD // 512                   # 4 windows of 512 per residue
F32 = mybir.dt.float32
BF16 = mybir.dt.bfloat16
ADD = mybir.AluOpType.add
MAX = mybir.AluOpType.max
MIN = mybir.AluOpType.min
MULT = mybir.AluOpType.mult

_COMPILED = None


def _quant_weight_int(w):
    """Integer levels + per-channel step of reference fake_quant_weight."""
    red = tuple(range(1, w.ndim))
    s = np.maximum(
        np.float32(GAMMA) * np.max(np.abs(w), axis=red, keepdims=True),
        np.float32(EPS),
    ).astype(np.float32)
    step = (s / np.float32(QMAX)).astype(np.float32)
    wc = np.clip(w, -s, s).astype(np.float32)
    wi = np.round((wc / step).astype(np.float32)).astype(np.float32)
    return wi, step.reshape(w.shape[0])


def _build_program():
    nc = bacc.Bacc(
        "TRN2",
        target_bir_lowering=False,
        debug=False,
        enable_asserts=False,
        num_devices=N_CORES,
    )

    xd = nc.dram_tensor("xd", [BPC, CIN, SF, TD], F32, kind="ExternalInput")
    w1t = nc.dram_tensor("w1t", [CIN, COUT], BF16, kind="ExternalInput")
    w2d = nc.dram_tensor("w2d", [128, 4 * KW * 128], BF16, kind="ExternalInput")
    beta = nc.dram_tensor("beta", [COUT], F32, kind="ExternalInput")
    gout = nc.dram_tensor("gout", [COUT], F32, kind="ExternalInput")
    outd = nc.dram_tensor("out", [BPC, COUT, TOUT], F32, kind="ExternalOutput")

    C = MAGIC
    pe_chain = []

    with TileContext(nc) as tc:
        with (
            tc.tile_pool(name="wpool", bufs=1) as wpool,
            tc.tile_pool(name="xraw", bufs=1) as xraw_pool,
            tc.tile_pool(name="xtmp", bufs=1) as xtmp_pool,
            tc.tile_pool(name="xip", bufs=2) as xi_pool,
            tc.tile_pool(name="yqp", bufs=2) as yq_pool,
            tc.tile_pool(name="qch", bufs=6) as qch_pool,
            tc.tile_pool(name="outp", bufs=4) as out_pool,
            tc.tile_pool(name="pmm", bufs=2, space="PSUM") as pmm_pool,
            tc.tile_pool(name="pdw", bufs=2, space="PSUM") as pdw_pool,
        ):
            w1sb = wpool.tile([CIN, COUT], BF16)          # lhsT for main mm
            nc.sync.dma_start(w1sb, w1t.ap())
            w2sb = wpool.tile([128, 4 * KW * 128], BF16)  # 32 diag blocks
            nc.sync.dma_start(w2sb, w2d.ap())
            beta_sb = wpool.tile([128, 4], F32)
            nc.sync.dma_start(beta_sb, beta.ap().rearrange("(q p) -> p q", p=128))
            gout_sb = wpool.tile([128, 4], F32)
            nc.sync.dma_start(gout_sb, gout.ap().rearrange("(q p) -> p q", p=128))

            for b in range(BPC):
                # x already scaled to u-units (x/step1) and deinterleaved on host
                xr = xraw_pool.tile([CIN, T], F32, tag="xr")
                nc.sync.dma_start(xr, xd.ap()[b].rearrange("c r u -> c (r u)"))
                xrnd = xtmp_pool.tile([CIN, T], F32, tag="xrnd")
                nc.vector.tensor_scalar(xrnd, xr, C, -C, ADD, ADD)  # round to int
                xi = xi_pool.tile([CIN, T], BF16, tag="xi")
                nc.vector.tensor_scalar(xi, xrnd, -QMAX, QMAX, MAX, MIN)

                for cb in range(4):
                    yq = yq_pool.tile([128, T], BF16, tag="yq")
                    for g in range(8):  # groups of 1024 (r = g//2, half = g%2)
                        r, h = g // 2, g % 2
                        base = r * TD + h * 1024
                        pu = pmm_pool.tile([128, 1024], F32, tag="pu")
                        for w in range(2):
                            mm = nc.tensor.matmul(
                                pu[:, w * 512 : (w + 1) * 512],
                                w1sb[:, cb * 128 : (cb + 1) * 128],
                                xi[:, base + w * 512 : base + (w + 1) * 512],
                                start=True,
                                stop=True,
                            )
                            pe_chain.append(mm.ins)
                        # u + C = S*beta + C  (single fused affine on ACT)
                        tq = qch_pool.tile([128, 1024], F32, tag="tq")
                        nc.scalar.activation(
                            tq,
                            pu,
                            mybir.ActivationFunctionType.Copy,
                            bias=C,
                            scale=beta_sb[:, cb : cb + 1],
                        )
                        # subtract C (round done) and clamp low
                        t2 = qch_pool.tile([128, 1024], BF16, tag="t2")
                        nc.vector.tensor_scalar(t2, tq, -C, -QMAX, ADD, MAX)
                        # clamp high -> integer levels of yq, in bf16
                        nc.vector.tensor_scalar(
                            yq[:, base : base + 1024], t2, QMAX, None, MIN
                        )

                    # depthwise conv: window-pairs in double-buffered PSUM,
                    # k-outer within a pair (2 same-weight matmuls per load)
                    for ho in range(2):
                        n = 1024 if ho == 0 else TOUT - 1024
                        pd = pdw_pool.tile([128, 1024], F32, tag="pd")
                        for k in range(KW):
                            r, j = k % SF, k // SF
                            for w4 in range(2):
                                m = min(512, n - w4 * 512)
                                base = r * TD + (2 * ho + w4) * 512 + j
                                mm = nc.tensor.matmul(
                                    pd[:, w4 * 512 : w4 * 512 + m],
                                    w2sb[
                                        :,
                                        (cb * KW + k) * 128 : (cb * KW + k + 1) * 128,
                                    ],
                                    yq[:, base : base + m],
                                    start=(k == 0),
                                    stop=(k == KW - 1),
                                )
                                pe_chain.append(mm.ins)
                        osb = out_pool.tile([128, 1024], F32, tag="osb")
                        if ho % 2 == 0:
                            nc.vector.tensor_scalar(
                                osb[:, :n], pd[:, :n],
                                gout_sb[:, cb : cb + 1], None, MULT,
                            )
                        else:
                            nc.scalar.activation(
                                osb[:, :n], pd[:, :n],
                                mybir.ActivationFunctionType.Copy,
                                bias=0.0, scale=gout_sb[:, cb : cb + 1],
                            )
                        nc.sync.dma_start(
                            outd.ap()[
                                b,
                                cb * 128 : (cb + 1) * 128,
                                ho * 1024 : ho * 1024 + n,
                            ],
                            osb[:, :n],
                        )

        pass

    nc.compile()
    return nc


def _get_program():
    global _COMPILED
    if _COMPILED is None:
        _COMPILED = _build_program()
    return _COMPILED


def kernel(x, x_len, w1, w2, s1, s2):
    out, _res = run_with_results(x, x_len, w1, w2, s1, s2)
    return out


def run_with_results(x, x_len, w1, w2, s1, s2, trace=False, **trace_kwargs):
    x = np.asarray(x, dtype=np.float32)
    x_len = np.asarray(x_len, dtype=np.float32)
    w1 = np.asarray(w1, dtype=np.float32)
    w2 = np.asarray(w2, dtype=np.float32)
    s1f = np.maximum(np.float32(np.asarray(s1).reshape(())), np.float32(EPS))
    s2f = np.maximum(np.float32(np.asarray(s2).reshape(())), np.float32(EPS))

    # host-side weight quantization (integer levels + scales)
    w1i, step_w1 = _quant_weight_int(w1)   # [512, 80, 1] ints, [512]
    w1i = w1i[:, :, 0]
    w2i, step_w2 = _quant_weight_int(w2)   # [512, 1, 8] ints, [512]
    w2i = w2i[:, 0, :]

    step1 = np.float32(s1f / np.float32(QMAX))
    step2 = np.float32(s2f / np.float32(QMAX))

    # beta: u = S * beta + rounding;  y = step_w1*step1*S ; u = y*127/s2
    beta = (step_w1 * step1 * (np.float32(QMAX) / s2f)).astype(np.float32)
    # gout: out = gout * sum_k W2i*yq_int
    gout = (step_w2 * step2).astype(np.float32)

    w1t = np.ascontiguousarray(w1i.T).astype(ml_dtypes.bfloat16)  # [80, 512]
    w2dm = np.zeros((128, 4 * KW * 128), dtype=np.float32)
    p = np.arange(128)
    for cb in range(4):
        for k in range(KW):
            w2dm[p, (cb * KW + k) * 128 + p] = w2i[cb * 128 + p, k]
    w2dm = w2dm.astype(ml_dtypes.bfloat16)

    # x -> u-units (divide, matching reference xc/step), deinterleave by residue
    xs = (x / step1).astype(np.float32)
    xd = np.ascontiguousarray(xs.reshape(B, CIN, TD, SF).transpose(0, 1, 3, 2))

    nc = _get_program()
    in_maps = []
    for core in range(N_CORES):
        in_maps.append(
            {
                "xd": xd[core * BPC : (core + 1) * BPC],
                "w1t": w1t,
                "w2d": w2dm,
                "beta": beta,
                "gout": gout,
            }
        )

    res = bass_utils.run_bass_kernel_spmd(
        nc, in_maps, core_ids=list(range(N_CORES)), trace=trace, **trace_kwargs
    )
    out = np.concatenate([r["out"] for r in res.results], axis=0)
    return (out, np.floor(x_len / SF)), res


# revision 14
# speedup vs baseline: 1.2218x; 1.0029x over previous
"""Trainium2 Bass kernel for quantized Conv1dSubsampling (nn_Conv1dSubsampling).

Reference computation (per batch):
  xq  = fake_quant_act(x, s1, 8)                    # clamp +/- s1, round to 255-level grid
  y   = w1q @ xq                                    # 1x1 conv: [512,80] @ [80,T]
  yq  = fake_quant_act(y, s2, 8)
  out = depthwise_conv(yq, w2q, stride=4, k=8)      # [512, 2047]

Kernel strategy (8 cores, data-parallel over batch: 2 batches/core):
  - Exact integer formulation: quantized activations/weights are integer
    levels in [-127,127], exact in bf16.  Both matmuls run on the PE in bf16
    with exact fp32 PSUM accumulation (|sums| < 2^24).  Per-channel scales
    are applied in the PSUM-evacuation ops (per-partition scalar operands).
  - Rounding uses the fp32 magic constant C=1.5*2^23: fl(fl(v+C)-C) = RNE(v).
  - Depthwise conv = 8 PSUM-accumulating matmuls with diagonal weight
    matrices (tap k = r + 4j: out[:, t] += diag(W2i[:, k]) @ Z_r[:, t+j]) on
    time-deinterleaved data Z_r[c, u] = yq[c, 4u + r].  x arrives from the
    host pre-deinterleaved and pre-scaled, so every device op is contiguous.
"""

import ml_dtypes
import numpy as np

import concourse.bass as bass  # noqa: F401  (env import check)
import concourse.mybir as mybir
from concourse import bacc
from concourse import bass_utils
from concourse.tile import TileContext
from concourse.tile import add_dep_helper

# Problem shapes (hardcoded per contest contract).
B, CIN, T, COUT = 16, 80, 8192, 512
SF = 4
KW = 2 * SF                      # depthwise kernel width = 8
TOUT = (T - KW) // SF + 1        # 2047
N_CORES = 8
BPC = B // N_CORES               # batches per core = 2
EPS = 1e-5
GAMMA = 0.9
QMAX = 127.0
MAGIC = float(np.float32(1.5 * 2.0**23))  # 12582912.0
TD = T // SF                     # 2048 (deinterleaved length per residue)
NW = TD // 512                   # 4 windows of 512 per residue
F32 = mybir.dt.float32
BF16 = mybir.dt.bfloat16
ADD = mybir.AluOpType.add
MAX = mybir.AluOpType.max
MIN = mybir.AluOpType.min
MULT = mybir.AluOpType.mult

_COMPILED = None


def _quant_weight_int(w):
    """Integer levels + per-channel step of reference fake_quant_weight."""
    red = tuple(range(1, w.ndim))
    s = np.maximum(
        np.float32(GAMMA) * np.max(np.abs(w), axis=red, keepdims=True),
        np.float32(EPS),
    ).astype(np.float32)
    step = (s / np.float32(QMAX)).astype(np.float32)
    wc = np.clip(w, -s, s).astype(np.float32)
    wi = np.round((wc / step).astype(np.float32)).astype(np.float32)
    return wi, step.reshape(w.shape[0])


def _build_program():
    nc = bacc.Bacc(
        "TRN2",
        target_bir_lowering=False,
        debug=False,
        enable_asserts=False,
        num_devices=N_CORES,
    )

    xd = nc.dram_tensor("xd", [BPC, CIN, SF, TD], F32, kind="ExternalInput")
    w1t = nc.dram_tensor("w1t", [CIN, COUT], BF16, kind="ExternalInput")
    w2d = nc.dram_tensor("w2d", [128, 4 * KW * 128], BF16, kind="ExternalInput")
    beta = nc.dram_tensor("beta", [COUT], F32, kind="ExternalInput")
    gout = nc.dram_tensor("gout", [COUT], F32, kind="ExternalInput")
    outd = nc.dram_tensor("out", [BPC, COUT, TOUT], F32, kind="ExternalOutput")

    C = MAGIC
    pe_chain = []

    with TileContext(nc) as tc:
        with (
            tc.tile_pool(name="wpool", bufs=1) as wpool,
            tc.tile_pool(name="xraw", bufs=1) as xraw_pool,
            tc.tile_pool(name="xtmp", bufs=1) as xtmp_pool,
            tc.tile_pool(name="xip", bufs=2) as xi_pool,
            tc.tile_pool(name="yqp", bufs=2) as yq_pool,
            tc.tile_pool(name="qch", bufs=8) as qch_pool,
            tc.tile_pool(name="outp", bufs=4) as out_pool,
            tc.tile_pool(name="pmm", bufs=2, space="PSUM") as pmm_pool,
            tc.tile_pool(name="pdw", bufs=2, space="PSUM") as pdw_pool,
        ):
            w1sb = wpool.tile([CIN, COUT], BF16)          # lhsT for main mm
            nc.sync.dma_start(w1sb, w1t.ap())
            w2sb = wpool.tile([128, 4 * KW * 128], BF16)  # 32 diag blocks
            nc.sync.dma_start(w2sb, w2d.ap())
            beta_sb = wpool.tile([128, 4], F32)
            nc.sync.dma_start(beta_sb, beta.ap().rearrange("(q p) -> p q", p=128))
            gout_sb = wpool.tile([128, 4], F32)
            nc.sync.dma_start(gout_sb, gout.ap().rearrange("(q p) -> p q", p=128))

            warm = wpool.tile([128, 512], BF16)
            nc.vector.memset(warm, 0.0)
            pwarm = pmm_pool.tile([128, 512], F32, tag="pu")
            for i in range(24):
                nc.tensor.matmul(
                    warm[:, 0:128], warm, pwarm if False else warm[:, 0:512],
                    start=True, stop=True,
                ) if False else None
                nc.tensor.matmul(
                    pwarm, warm[:, 0:128], warm, start=(i == 0), stop=(i == 23)
                )

            for b in range(BPC):
                # x already scaled to u-units (x/step1) and deinterleaved on host
                xr = xraw_pool.tile([CIN, T], F32, tag="xr")
                nc.sync.dma_start(xr, xd.ap()[b].rearrange("c r u -> c (r u)"))
                xrnd = xtmp_pool.tile([CIN, T], F32, tag="xrnd")
                nc.vector.tensor_scalar(xrnd, xr, C, -C, ADD, ADD)  # round to int
                xi = xi_pool.tile([CIN, T], BF16, tag="xi")
                nc.vector.tensor_scalar(xi, xrnd, -QMAX, QMAX, MAX, MIN)

                for cb in range(4):
                    yq = yq_pool.tile([128, T], BF16, tag="yq")
                    for g in range(8):  # groups of 1024 (r = g//2, half = g%2)
                        r, h = g // 2, g % 2
                        base = r * TD + h * 1024
                        pu = pmm_pool.tile([128, 1024], F32, tag="pu")
                        for w in range(2):
                            mm = nc.tensor.matmul(
                                pu[:, w * 512 : (w + 1) * 512],
                                w1sb[:, cb * 128 : (cb + 1) * 128],
                                xi[:, base + w * 512 : base + (w + 1) * 512],
                                start=True,
                                stop=True,
                            )
                            pe_chain.append(mm.ins)
                        # u + C = S*beta + C  (single fused affine on ACT)
                        tq = qch_pool.tile([128, 1024], F32, tag="tq")
                        nc.scalar.activation(
                            tq,
                            pu,
                            mybir.ActivationFunctionType.Copy,
                            bias=C,
                            scale=beta_sb[:, cb : cb + 1],
                        )
                        # subtract C (round done) and clamp low
                        t2 = qch_pool.tile([128, 1024], BF16, tag="t2")
                        nc.vector.tensor_scalar(t2, tq, -C, -QMAX, ADD, MAX)
                        # clamp high -> integer levels of yq, in bf16
                        nc.vector.tensor_scalar(
                            yq[:, base : base + 1024], t2, QMAX, None, MIN
                        )

                    # depthwise conv: window-pairs in double-buffered PSUM,
                    # k-outer within a pair (2 same-weight matmuls per load)
                    for ho in range(2):
                        n = 1024 if ho == 0 else TOUT - 1024
                        pd = pdw_pool.tile([128, 1024], F32, tag="pd")
                        for k in range(KW):
                            r, j = k % SF, k // SF
                            for w4 in range(2):
                                m = min(512, n - w4 * 512)
                                base = r * TD + (2 * ho + w4) * 512 + j
                                mm = nc.tensor.matmul(
                                    pd[:, w4 * 512 : w4 * 512 + m],
                                    w2sb[
                                        :,
                                        (cb * KW + k) * 128 : (cb * KW + k + 1) * 128,
                                    ],
                                    yq[:, base : base + m],
                                    start=(k == 0),
                                    stop=(k == KW - 1),
                                )
                                pe_chain.append(mm.ins)
                        osb = out_pool.tile([128, 1024], F32, tag="osb")
                        if ho % 2 == 0:
                            nc.vector.tensor_scalar(
                                osb[:, :n], pd[:, :n],
                                gout_sb[:, cb : cb + 1], None, MULT,
                            )
                        else:
                            nc.scalar.activation(
                                osb[:, :n], pd[:, :n],
                                mybir.ActivationFunctionType.Copy,
                                bias=0.0, scale=gout_sb[:, cb : cb + 1],
                            )
                        nc.sync.dma_start(
                            outd.ap()[
                                b,
                                cb * 128 : (cb + 1) * 128,
                                ho * 1024 : ho * 1024 + n,
                            ],
                            osb[:, :n],
                        )

        pass

    nc.compile()
    return nc


def _get_program():
    global _COMPILED
    if _COMPILED is None:
        _COMPILED = _build_program()
    return _COMPILED


def kernel(x, x_len, w1, w2, s1, s2):
    out, _res = run_with_results(x, x_len, w1, w2, s1, s2)
    return out


def run_with_results(x, x_len, w1, w2, s1, s2, trace=False, **trace_kwargs):
    x = np.asarray(x, dtype=np.float32)
    x_len = np.asarray(x_len, dtype=np.float32)
    w1 = np.asarray(w1, dtype=np.float32)
    w2 = np.asarray(w2, dtype=np.float32)
    s1f = np.maximum(np.float32(np.asarray(s1).reshape(())), np.float32(EPS))
    s2f = np.maximum(np.float32(np.asarray(s2).reshape(())), np.float32(EPS))

    # host-side weight quantization (integer levels + scales)
    w1i, step_w1 = _quant_weight_int(w1)   # [512, 80, 1] ints, [512]
    w1i = w1i[:, :, 0]
    w2i, step_w2 = _quant_weight_int(w2)   # [512, 1, 8] ints, [512]
    w2i = w2i[:, 0, :]

    step1 = np.float32(s1f / np.float32(QMAX))
    step2 = np.float32(s2f / np.float32(QMAX))

    # beta: u = S * beta + rounding;  y = step_w1*step1*S ; u = y*127/s2
    beta = (step_w1 * step1 * (np.float32(QMAX) / s2f)).astype(np.float32)
    # gout: out = gout * sum_k W2i*yq_int
    gout = (step_w2 * step2).astype(np.float32)

    w1t = np.ascontiguousarray(w1i.T).astype(ml_dtypes.bfloat16)  # [80, 512]
    w2dm = np.zeros((128, 4 * KW * 128), dtype=np.float32)
    p = np.arange(128)
    for cb in range(4):
        for k in range(KW):
            w2dm[p, (cb * KW + k) * 128 + p] = w2i[cb * 128 + p, k]
    w2dm = w2dm.astype(ml_dtypes.bfloat16)

    # x -> u-units (divide, matching reference xc/step), deinterleave by residue
    xs = (x / step1).astype(np.float32)
    xd = np.ascontiguousarray(xs.reshape(B, CIN, TD, SF).transpose(0, 1, 3, 2))

    nc = _get_program()
    in_maps = []
    for core in range(N_CORES):
        in_maps.append(
            {
                "xd": xd[core * BPC : (core + 1) * BPC],
                "w1t": w1t,
                "w2d": w2dm,
                "beta": beta,
                "gout": gout,
            }
        )

    res = bass_utils.run_bass_kernel_spmd(
        nc, in_maps, core_ids=list(range(N_CORES)), trace=trace, **trace_kwargs
    )
    out = np.concatenate([r["out"] for r in res.results], axis=0)
    return (out, np.floor(x_len / SF)), res
